# revision 46
# baseline (speedup 1.0000x reference)
"""Tri-quadratic (order-3) tensor-product B-spline evaluation at 2M points,
computed on 8 NeuronCores.

Pipeline
--------
Host (threaded numpy, ~50ms): quantize each coordinate to 16 bits
(6-bit knot-interval index + 10-bit fraction, q = floor(u*62*1024)) —
6 bytes/point of H2D traffic, the information floor for this accuracy.
Device (Bass, SPMD over 8 cores): per 8192-point batch the kernel
unpacks q, computes the quadratic Cox-de-Boor basis in f32, derives a
15-bit overlapping-tile id (4x4x4 coefficient tiles with stride 2), DMA-
shuffles the ids into the GPSIMD gather wrap layout, and issues one
512-byte dma_gather descriptor per point (split into 1024-index calls —
the SWDGE descriptor ring holds 128 in-flight entries) against an
overlapping-tile fp16 table baked into the NEFF as a Const tensor (it
never crosses the host link). The intra-tile offset is folded into
per-axis 4-tap weight vectors and the 192-tap tensor-product contraction
runs as three fp16 multiply + masked tensor_tensor_scan (segmented-sum,
f32 scan state) stages. The result is packed on-device to 3x10-bit
fixed point (one uint32 per point, 4 bytes/point D2H) using a per-tile
gain stored in the gather record's padding; the host dequantizes with
its own copy of the gain table in parallel with the per-shard fetches.

Wall-clock is dominated by the axon-tunneled host<->device link
(~45 MB/s); device execution itself is ~1.2 ms/core (cost model).
The coefficient table depends only on `coeff`; the compiled program and
jitted dispatcher are cached across calls keyed on the coeff bytes.
If anything about the device path fails, a numpy fallback (matches the
reference to ~1e-5) computes the result instead.
"""

import numpy as np

F32 = np.float32
NP_TOTAL = 2_000_000
N_CORES = 8
P = 128            # SBUF partitions
FPTS = 64          # points per partition per batch
BATCH = P * FPTS   # 8192 points per batch
NB = 31            # batches per core per chunk
NCHUNK = 1         # single dispatch; 31*8192 = 253952 >= 250000/core
CORE_PTS = NB * BATCH
NGRID = 64
NSEG = 62
NTILE = 31         # overlapping 4^3 tiles with stride 2 per axis
TROWS = NTILE ** 3  # 29791 table rows
TREC = 256         # fp16 elems per table row (192 payload + 64 pad) = 512B
GATHER_SPLIT = 8   # sub-gathers per batch; 1024 idxs each keeps the
                   # 128-deep SWDGE descriptor ring from overflowing


# ---------------------------------------------------------------------------
# Host-side packing
# ---------------------------------------------------------------------------

def _quantize(uvw):
    """uvw [3, N] f32 -> q uint16 [3, N]: q = floor(clip(u*62)*1024), i.e.
    interval index in bits 10..15, 10-bit fraction below (dequant /1024)."""
    t = uvw * F32(63488.0)                      # 62 * 1024
    np.clip(t, F32(0.0), F32(63487.0), out=t)   # i <= 61, fq <= 1023
    return t.astype(np.uint16)


OUT_BITS = 10       # output fixed-point bits per component (8 or 10; 8
                    # saves 2MB of D2H but measured 2.07e-2 rel err — over
                    # the 2e-2 gate — so 10 it is)
OUT_MARGIN = 124.0 if OUT_BITS == 8 else 505.0  # full-scale w/ fp16 headroom
OUT_OFF = 128.5 if OUT_BITS == 8 else 512.5     # trunc-to-round bias + offset


def _build_table(coeff):
    """fp16 table [TROWS, TREC]; row (qu,qv,qw) = coeff[:, 2qu:2qu+4, ...]
    laid out [c, x, y, z] in the first 192 elems. Element 192 holds the
    per-tile output quantization gain A = OUT_MARGIN / max|window|."""
    sw = np.lib.stride_tricks.sliding_window_view(coeff, (4, 4, 4), axis=(1, 2, 3))
    sw = sw[:, ::2, ::2, ::2]                        # [3,31,31,31,4,4,4]
    tbl = np.zeros((TROWS, TREC), np.float16)
    payload = np.moveaxis(sw, 0, 3).reshape(TROWS, 192)
    tbl[:, :192] = payload
    S = np.maximum(np.abs(payload).max(axis=1), np.float32(1e-3))
    gain = (np.float32(OUT_MARGIN) / S).astype(np.float16)
    tbl[:, 192] = gain
    return tbl, gain


def _pack_inputs(uvw):
    """-> [Q_chunk0, ...] each [8*3, NB, P, FPTS] u16 (per-core block =
    [3, NB, P, F], plain memcpy of the padded quantized stream),
    tid int32 [8, NP/8] (for output dequant). Per-core work in threads."""
    from concurrent.futures import ThreadPoolExecutor
    Nc = NP_TOTAL // N_CORES
    tot = NCHUNK * CORE_PTS
    chunks = _STATE.get("packbufs")
    if chunks is None:
        chunks = [np.empty((N_CORES * 3, NB, P, FPTS), np.uint16)
                  for _ in range(NCHUNK)]
        _STATE["packbufs"] = chunks

    def pack_core(c):
        t = uvw[:, c * Nc:(c + 1) * Nc] * F32(63488.0)
        np.clip(t, F32(0.0), F32(63487.0), out=t)
        q = t.astype(np.uint16)
        for k in range(NCHUNK):
            dst = chunks[k][c * 3:(c + 1) * 3].reshape(3, CORE_PTS)
            lo = k * CORE_PTS
            n = max(0, min(CORE_PTS, Nc - lo))
            dst[:, :n] = q[:, lo:lo + n]
            if n < CORE_PTS:
                dst[:, n:] = 0

    with ThreadPoolExecutor(N_CORES) as ex:
        list(ex.map(pack_core, range(N_CORES)))
    # tids are derived lazily (inside the D2H-overlapped fetch threads)
    # from the packed q planes; hand back the chunk views needed for it.
    return chunks, None


def _unpack_chunk(out_arr, k, chunks, inv_tab, out, ex):
    """Fetch + dequant one chunk's sharded int32 output into `out`.
    Per-point tile ids (for the dequant gain) are derived here from the
    packed q planes, overlapped with the D2H transfers."""
    Nc = NP_TOTAL // N_CORES
    shards = sorted(out_arr.addressable_shards, key=lambda s: s.index[0].start or 0)

    def fetch(args):
        c, sh = args
        base = k * CORE_PTS
        count = min(CORE_PTS, Nc - base)
        if count <= 0:
            _ = np.asarray(sh.data)  # still drain the transfer
            return
        qv = chunks[k][c * 3:(c + 1) * 3].reshape(3, CORE_PTS)[:, :count]
        tid_c = ((qv[0] >> 11).astype(np.int32) * 961
                 + (qv[1] >> 11).astype(np.int32) * 31
                 + (qv[2] >> 11).astype(np.int32))
        inv = inv_tab[tid_c]
        sl = out[:, c * Nc + base:c * Nc + base + count]
        if OUT_BITS == 8:
            raw = np.asarray(sh.data).reshape(CORE_PTS, 3)[:count]
            v = raw.astype(np.int32) - 128
            sl[0] = v[:, 0].astype(np.float32) * inv
            sl[1] = v[:, 1].astype(np.float32) * inv
            sl[2] = v[:, 2].astype(np.float32) * inv
        else:
            raw = np.asarray(sh.data).reshape(CORE_PTS)[:count]   # i32
            sl[0] = ((raw & 1023) - 512).astype(np.float32) * inv
            sl[1] = (((raw >> 10) & 1023) - 512).astype(np.float32) * inv
            sl[2] = (((raw >> 20) & 1023) - 512).astype(np.float32) * inv

    list(ex.map(fetch, enumerate(shards)))


# ---------------------------------------------------------------------------
# Device program
# ---------------------------------------------------------------------------

def _build_program(table):
    import concourse.bass as bass
    import concourse.tile as tile
    from concourse import bacc, mybir
    from contextlib import ExitStack

    dt = mybir.dt
    op = mybir.AluOpType
    F = FPTS

    nc = bacc.Bacc("TRN2", target_bir_lowering=False, debug=False)
    q_d = nc.dram_tensor("qpk", [3, NB, P, F], dt.uint16, kind="ExternalInput")
    if OUT_BITS == 8:
        o_d = nc.dram_tensor("xyzo", [NB, P, 3 * F], dt.uint8, kind="ExternalOutput")
    else:
        o_d = nc.dram_tensor("xyzo", [NB, P, F], dt.int32, kind="ExternalOutput")
    t_d = nc.inline_tensor(table, name="tbl")

    with nc.allow_low_precision(reason="fp16 partials; scan state is f32"):
        with tile.TileContext(nc) as tc:
            with ExitStack() as ctx:
                cpool = ctx.enter_context(tc.tile_pool(name="c", bufs=1))
                pool = ctx.enter_context(tc.tile_pool(name="p", bufs=2))

                # constant scan mask: repeating [0,1,1,1] fp16
                mask = cpool.tile([P, F * 192], dt.float16, tag="mask")
                nc.vector.memset(mask[:], 1.0)
                nc.vector.memset(
                    mask[:].rearrange("p (s z) -> p s z", z=4)[:, :, 0], 0.0)

                for b in range(NB):
                    qt = pool.tile([P, 3, F], dt.uint16, tag="q")
                    for d in range(3):
                        nc.sync.dma_start(qt[:, d, :], q_d.ap()[d, b])

                    # ---- unpack q -> i, f, du (f32), all 3 dims at once
                    i16 = pool.tile([P, 3, F], dt.uint16, tag="i16")
                    f16i = pool.tile([P, 3, F], dt.uint16, tag="f16i")
                    d16 = pool.tile([P, 3, F], dt.uint16, tag="d16")
                    nc.vector.tensor_scalar(i16[:], qt[:], 10, None, op.logical_shift_right)
                    nc.vector.tensor_scalar(f16i[:], qt[:], 1023, None, op.bitwise_and)
                    nc.vector.tensor_scalar(d16[:], qt[:], 10, 1, op.logical_shift_right, op.bitwise_and)
                    iuf = pool.tile([P, 3, F], dt.float32, tag="iuf")
                    ff = pool.tile([P, 3, F], dt.float32, tag="ff")
                    duf = pool.tile([P, 3, F], dt.float32, tag="duf")
                    nc.vector.tensor_copy(iuf[:], i16[:])
                    nc.vector.tensor_copy(ff[:], f16i[:])
                    nc.vector.tensor_copy(duf[:], d16[:])
                    nc.vector.tensor_scalar(ff[:], ff[:], float(1.0 / 1024.0), None, op.mult)

                    # ---- tile ids (f32 exact) -> int16, shuffled into the
                    # [16, i//16] gather wrap (i = g*128 + p, p = 16j + qq)
                    iu2 = pool.tile([P, 3, F], dt.float32, tag="iu2")
                    nc.vector.tensor_tensor(iu2[:], iuf[:], duf[:], op.subtract)
                    wh = pool.tile([P, F], dt.float32, tag="wh")
                    t1 = pool.tile([P, F], dt.float32, tag="t1")
                    tidf = pool.tile([P, F], dt.float32, tag="tidf")
                    nc.vector.tensor_scalar(wh[:], iu2[:, 2, :], 0.5, None, op.mult)
                    nc.vector.scalar_tensor_tensor(t1[:], iu2[:, 1, :], 15.5, wh[:], op.mult, op.add)
                    nc.vector.scalar_tensor_tensor(tidf[:], iu2[:, 0, :], 480.5, t1[:], op.mult, op.add)
                    tid16 = pool.tile([P, F], dt.int16, tag="tid16")
                    nc.vector.tensor_copy(tid16[:], tidf[:])
                    idx = pool.tile([P, BATCH // 16], dt.int16, tag="idx")
                    idx_v = idx[:].rearrange("p (g j) -> p g j", j=8)
                    for j in range(8):
                        nc.sync.dma_start(idx_v[0:16, :, j], tid16[16 * j:16 * (j + 1), :])
                    for j in range(1, 8):
                        nc.sync.dma_start(idx[16 * j:16 * (j + 1), :], idx[0:16, :])

                    win = pool.tile([P, F, TREC], dt.float16, tag="win")
                    S = GATHER_SPLIT
                    sub = BATCH // S
                    for k in range(S):
                        nc.gpsimd.dma_gather(
                            win[:, k * (F // S):(k + 1) * (F // S), :],
                            t_d.ap(),
                            idx[:, k * (sub // 16):(k + 1) * (sub // 16)],
                            sub, sub, TREC)

                    # ---- basis N0/N1/N2 -> nub [P, 3, F, 5] rows 1..3
                    rd0 = pool.tile([P, 3, F], dt.float32, tag="rd0")
                    rd2 = pool.tile([P, 3, F], dt.float32, tag="rd2")
                    nc.vector.tensor_scalar(rd0[:], iuf[:], 0.0, 0.5, op.is_equal, op.mult)
                    nc.vector.tensor_scalar(rd0[:], rd0[:], 0.5, None, op.add)
                    nc.vector.tensor_scalar(rd2[:], iuf[:], 61.0, 0.5, op.is_equal, op.mult)
                    nc.vector.tensor_scalar(rd2[:], rd2[:], 0.5, None, op.add)
                    omf = pool.tile([P, 3, F], dt.float32, tag="omf")
                    nc.vector.tensor_scalar(omf[:], ff[:], -1.0, 1.0, op.mult, op.add)
                    sq = pool.tile([P, 3, F], dt.float32, tag="sq")
                    nub = pool.tile([P, 3, F, 5], dt.float32, tag="nub")
                    nc.vector.memset(nub[:], 0.0)
                    nc.vector.tensor_tensor(sq[:], omf[:], omf[:], op.mult)
                    nc.vector.tensor_tensor(nub[:, :, :, 1], sq[:], rd0[:], op.mult)
                    nc.vector.tensor_tensor(sq[:], ff[:], ff[:], op.mult)
                    nc.vector.tensor_tensor(nub[:, :, :, 3], sq[:], rd2[:], op.mult)
                    nc.vector.tensor_tensor(sq[:], nub[:, :, :, 1], nub[:, :, :, 3], op.add)
                    nc.vector.tensor_scalar(nub[:, :, :, 2], sq[:], -1.0, 1.0, op.mult, op.add)

                    # ---- extended 4-tap weights nup fp16 [P, 3, F, 4]
                    diff = pool.tile([P, 3, F, 4], dt.float32, tag="diff")
                    nup = pool.tile([P, 3, F, 4], dt.float16, tag="nup")
                    nc.vector.tensor_tensor(diff[:], nub[:, :, :, 0:4], nub[:, :, :, 1:5], op.subtract)
                    nc.vector.tensor_tensor(diff[:], diff[:], duf[:].broadcast_to([P, 3, F, 4]), op.mult)
                    nc.vector.tensor_tensor(nup[:], diff[:], nub[:, :, :, 1:5], op.add)

                    # ---- contraction: [c,x,y,z] win * nw -> scan z -> * nv
                    #      -> scan y -> * nu -> scan x
                    prodz = pool.tile([P, F * 192], dt.float16, tag="prodz")
                    nc.vector.tensor_tensor(
                        prodz[:].rearrange("p (f s z) -> p f s z", s=48, z=4),
                        win[:].rearrange("p f (s z) -> p f s z", z=4)[:, :, 0:48, :],
                        nup[:, 2, :, :].unsqueeze(2).broadcast_to([P, F, 48, 4]),
                        op.mult)
                    nc.vector.tensor_tensor_scan(
                        prodz[:], mask[:], prodz[:], 0.0, op.mult, op.add)
                    prody = pool.tile([P, F * 48], dt.float16, tag="prody")
                    prody_v = prody[:].rearrange("p (f c x y) -> p f c x y",
                                                 c=3, x=4, y=4)
                    zscan_v = prodz[:].rearrange("p (f c x y z) -> p f c x y z",
                                                 c=3, x=4, y=4, z=4)
                    for cc in range(3):
                        nc.vector.tensor_tensor(
                            prody_v[:, :, cc, :, :],
                            zscan_v[:, :, cc, :, :, 3],
                            nup[:, 1, :, :].unsqueeze(2).broadcast_to([P, F, 4, 4]),
                            op.mult)
                    nc.vector.tensor_tensor_scan(
                        prody[:], mask[:, 0:F * 48], prody[:], 0.0, op.mult, op.add)
                    prodx = pool.tile([P, F * 12], dt.float16, tag="prodx")
                    nc.vector.tensor_tensor(
                        prodx[:].rearrange("p (f c x) -> p f c x", c=3, x=4),
                        prody[:].rearrange("p (f c x y) -> p f c x y",
                                           c=3, x=4, y=4)[:, :, :, :, 3],
                        nup[:, 0, :, :].unsqueeze(2).broadcast_to([P, F, 3, 4]),
                        op.mult)
                    nc.vector.tensor_tensor_scan(
                        prodx[:], mask[:, 0:F * 12], prodx[:], 0.0, op.mult, op.add)
                    # ---- fixed-point pack: v = trunc(x*A + OUT_OFF)
                    af = pool.tile([P, F], dt.float32, tag="af")
                    nc.vector.tensor_copy(af[:], win[:, :, 192])
                    xyzf = pool.tile([P, F, 3], dt.float32, tag="xyzf")
                    nc.vector.tensor_copy(
                        xyzf[:],
                        prodx[:].rearrange("p (f c x) -> p f c x", c=3, x=4)[:, :, :, 3])
                    nc.vector.tensor_tensor(
                        xyzf[:], xyzf[:], af[:].broadcast_to([P, F, 3]), op.mult)
                    if OUT_BITS == 8:
                        v8 = pool.tile([P, F * 3], dt.uint8, tag="v8")
                        nc.vector.tensor_scalar(
                            v8[:].rearrange("p (f c) -> p f c", c=3),
                            xyzf[:], OUT_OFF, None, op.add)
                        nc.sync.dma_start(o_d.ap()[b], v8[:])
                    else:
                        vi = pool.tile([P, F, 3], dt.int32, tag="vi")
                        nc.vector.tensor_scalar(vi[:], xyzf[:], OUT_OFF, None, op.add)
                        s1 = pool.tile([P, F], dt.int32, tag="s1")
                        s2 = pool.tile([P, F], dt.int32, tag="s2")
                        nc.vector.tensor_scalar(s1[:], vi[:, :, 1], 10, None, op.logical_shift_left)
                        nc.vector.tensor_scalar(s2[:], vi[:, :, 2], 20, None, op.logical_shift_left)
                        out32 = pool.tile([P, F], dt.int32, tag="out32")
                        nc.vector.tensor_tensor(out32[:], vi[:, :, 0], s1[:], op.bitwise_or)
                        nc.vector.tensor_tensor(out32[:], out32[:], s2[:], op.bitwise_or)
                        nc.sync.dma_start(o_d.ap()[b], out32[:])

    nc.compile()
    return nc


# ---------------------------------------------------------------------------
# Dispatch (cached jitted shard_map over 8 cores)
# ---------------------------------------------------------------------------

_STATE = {"key": None, "fn": None, "fail": False}


def _make_dispatch(nc):
    import jax
    import jax.numpy as jnp
    from jax.sharding import Mesh, PartitionSpec
    from jax.experimental.shard_map import shard_map
    from concourse import mybir
    from concourse.bass2jax import (_bass_exec_p, partition_id_tensor,
                                    install_neuronx_cc_hook)

    install_neuronx_cc_hook()
    try:
        jax.config.update("jax_compilation_cache_dir", "/tmp/jax_comp_cache")
        jax.config.update("jax_persistent_cache_min_compile_time_secs", 1.0)
    except Exception:
        pass

    in_names, out_names, out_avals = [], [], []
    partition_name = (nc.partition_id_tensor.name
                      if nc.partition_id_tensor is not None else None)
    for alloc in nc.m.functions[0].allocations:
        if not isinstance(alloc, mybir.MemoryLocationSet):
            continue
        name = alloc.memorylocations[0].name
        if alloc.kind == "ExternalInput":
            if name != partition_name:
                in_names.append(name)
        elif alloc.kind == "ExternalOutput":
            out_names.append(name)
            out_avals.append(jax.core.ShapedArray(
                tuple(alloc.tensor_shape), mybir.dt.np(alloc.dtype)))
    n_params = len(in_names)
    all_names = list(in_names) + list(out_names)
    if partition_name is not None:
        all_names.append(partition_name)

    def _body(*args):
        operands = list(args)
        if partition_name is not None:
            operands.append(partition_id_tensor())
        outs = _bass_exec_p.bind(
            *operands,
            out_avals=tuple(out_avals),
            in_names=tuple(all_names),
            out_names=tuple(out_names),
            lowering_input_output_aliases=(),
            sim_require_finite=False,
            sim_require_nnan=False,
            nc=nc,
        )
        return tuple(outs)

    n_out = len(out_names)
    devices = jax.devices()[:N_CORES]
    mesh = Mesh(np.asarray(devices), ("core",))
    sharding = jax.sharding.NamedSharding(mesh, PartitionSpec("core"))
    fn = jax.jit(shard_map(
        _body, mesh=mesh,
        in_specs=(PartitionSpec("core"),) * (n_params + n_out),
        out_specs=(PartitionSpec("core"),) * n_out,
        check_rep=False),
        donate_argnums=tuple(range(n_params, n_params + n_out)))

    def make_outbufs():
        return [jax.device_put(
            np.zeros(tuple([N_CORES * av.shape[0]] + list(av.shape[1:])),
                     av.dtype), sharding)
                for av in out_avals]

    return fn, in_names, out_names, make_outbufs


def _get_dispatch(coeff):
    key = hash(coeff.tobytes())
    if _STATE["key"] == key:
        return _STATE["fn"]
    table, gain = _build_table(coeff)
    nc = _build_program(table)
    fn = _make_dispatch(nc)
    _STATE["key"] = key
    _STATE["fn"] = fn
    _STATE["nc"] = nc
    _STATE["inv_tab"] = (np.float32(1.0)
                         / gain.astype(np.float32)).astype(np.float32)
    _STATE["outbufs"] = None
    return fn


# ---------------------------------------------------------------------------
# Host fallback (matches reference to ~1e-5)
# ---------------------------------------------------------------------------

def _basis_f32(X):
    X = np.maximum(X, F32(1e-14)).astype(F32)
    t = (X * F32(62.0)).astype(F32)
    C = F32(2 ** 23)
    r = ((t + C) - C).astype(F32)
    g = (t > r).astype(F32)
    i = (r + g - F32(1.0)).astype(F32)
    np.clip(i, F32(0.0), F32(61.0), out=i)
    f = (t - i).astype(F32)
    omf = (F32(1.0) - f).astype(F32)
    eq0 = (i == F32(0.0)).astype(F32)
    eq61 = (i == F32(61.0)).astype(F32)
    rD31 = (eq0 * F32(0.5) + F32(0.5)).astype(F32)
    rD42 = (eq61 * F32(0.5) + F32(0.5)).astype(F32)
    N0 = (omf * omf * rD31).astype(F32)
    N2 = (f * f * rD42).astype(F32)
    N1 = ((F32(1.0) - N0) - N2).astype(F32)
    return i.astype(np.int64), N0, N1, N2


def _spline_eval(uvw, coeff, chunk=262144):
    iu, NU0, NU1, NU2 = _basis_f32(uvw[0])
    iv, NV0, NV1, NV2 = _basis_f32(uvw[1])
    iw, NW0, NW1, NW2 = _basis_f32(uvw[2])
    NU = (NU0, NU1, NU2)
    NV = (NV0, NV1, NV2)
    NW = (NW0, NW1, NW2)
    cf = np.ascontiguousarray(coeff.reshape(3, -1))
    V = np.lib.stride_tricks.sliding_window_view(cf, 3, axis=1)
    base = (iu.astype(np.int32) * np.int32(NGRID * NGRID)
            + iv.astype(np.int32) * np.int32(NGRID) + iw.astype(np.int32))
    N = uvw.shape[1]
    out = np.empty((3, N), dtype=F32)
    for s in range(0, N, chunk):
        e = min(s + chunk, N)
        b = base[s:e]
        acc = np.zeros((3, e - s), dtype=F32)
        for ii in range(3):
            for jj in range(3):
                idx = b + np.int32(ii * NGRID * NGRID + jj * NGRID)
                G = V[:, idx, :]
                wuv = NU[ii][s:e] * NV[jj][s:e]
                w0 = wuv * NW[0][s:e]
                w1 = wuv * NW[1][s:e]
                w2 = wuv * NW[2][s:e]
                acc += G[:, :, 0] * w0 + G[:, :, 1] * w1 + G[:, :, 2] * w2
        out[:, s:e] = acc
    return out


# ---------------------------------------------------------------------------
# Entry point
# ---------------------------------------------------------------------------

def kernel(uvw, knotx, knoty, knotz, coeff, order):
    uvw = np.asarray(uvw, dtype=np.float32)
    coeff = np.asarray(coeff, dtype=np.float32)
    if not _STATE["fail"]:
        try:
            from concurrent.futures import ThreadPoolExecutor
            fn, in_names, out_names, make_outbufs = _get_dispatch(coeff)
            chunks, _ = _pack_inputs(uvw)
            outbufs = _STATE.get("outbufs")
            if outbufs is None:
                outbufs = [make_outbufs() for _ in range(NCHUNK)]
            # dispatch all chunks asynchronously, then drain in order so
            # chunk k's D2H overlaps chunk k+1's H2D/exec
            outs = [fn(chunks[k], *outbufs[k]) for k in range(NCHUNK)]
            _STATE["outbufs"] = [list(o) for o in outs]
            oi = out_names.index("xyzo")
            out = np.empty((3, NP_TOTAL), np.float32)
            with ThreadPoolExecutor(N_CORES) as ex:
                for k in range(NCHUNK):
                    _unpack_chunk(outs[k][oi], k, chunks,
                                  _STATE["inv_tab"], out, ex)
            return out
        except Exception:
            import traceback
            traceback.print_exc()
            _STATE["fail"] = True
            _STATE["outbufs"] = None
    return _spline_eval(uvw, coeff).astype(np.float32)


# revision 49
# speedup vs baseline: 1.4680x; 1.4680x over previous
"""Tri-quadratic (order-3) tensor-product B-spline evaluation at 2M points,
computed on 8 NeuronCores.

Pipeline
--------
Host (threaded numpy, ~50ms): quantize each coordinate to 16 bits
(6-bit knot-interval index + 10-bit fraction, q = floor(u*62*1024)) —
6 bytes/point of H2D traffic, the information floor for this accuracy.
Device (Bass, SPMD over 8 cores): per 8192-point batch the kernel
unpacks q, computes the quadratic Cox-de-Boor basis in f32, derives a
15-bit overlapping-tile id (4x4x4 coefficient tiles with stride 2), DMA-
shuffles the ids into the GPSIMD gather wrap layout, and issues one
512-byte dma_gather descriptor per point (split into 1024-index calls —
the SWDGE descriptor ring holds 128 in-flight entries) against an
overlapping-tile fp16 table baked into the NEFF as a Const tensor (it
never crosses the host link). The intra-tile offset is folded into
per-axis 4-tap weight vectors and the 192-tap tensor-product contraction
runs as three fp16 multiply + masked tensor_tensor_scan (segmented-sum,
f32 scan state) stages. The result is packed on-device to 3x10-bit
fixed point (one uint32 per point, 4 bytes/point D2H) using a per-tile
gain stored in the gather record's padding; the host dequantizes with
its own copy of the gain table in parallel with the per-shard fetches.

Wall-clock is dominated by the axon-tunneled host<->device link
(~45 MB/s); device execution itself is ~1.2 ms/core (cost model).
The coefficient table depends only on `coeff`; the compiled program and
jitted dispatcher are cached across calls keyed on the coeff bytes.
If anything about the device path fails, a numpy fallback (matches the
reference to ~1e-5) computes the result instead.
"""

import numpy as np

F32 = np.float32
NP_TOTAL = 2_000_000
N_CORES = 8
P = 128            # SBUF partitions
FPTS = 64          # points per partition per batch
BATCH = P * FPTS   # 8192 points per batch
NB = 31            # batches per core per chunk
NCHUNK = 1         # single dispatch; 31*8192 = 253952 >= 250000/core
CORE_PTS = NB * BATCH
NGRID = 64
NSEG = 62
NTILE = 31         # overlapping 4^3 tiles with stride 2 per axis
TROWS = NTILE ** 3  # 29791 table rows
TREC = 256         # fp16 elems per table row (192 payload + 64 pad) = 512B
GATHER_SPLIT = 8   # sub-gathers per batch; 1024 idxs each keeps the
                   # 128-deep SWDGE descriptor ring from overflowing


# ---------------------------------------------------------------------------
# Host-side packing
# ---------------------------------------------------------------------------

def _quantize(uvw):
    """uvw [3, N] f32 -> q uint16 [3, N]: q = floor(clip(u*62)*1024), i.e.
    interval index in bits 10..15, 10-bit fraction below (dequant /1024)."""
    t = uvw * F32(63488.0)                      # 62 * 1024
    np.clip(t, F32(0.0), F32(63487.0), out=t)   # i <= 61, fq <= 1023
    return t.astype(np.uint16)


OUT_BITS = 10       # output fixed-point bits per component (8 or 10; 8
                    # saves 2MB of D2H but measured 2.07e-2 rel err — over
                    # the 2e-2 gate — so 10 it is)
OUT_MARGIN = 124.0 if OUT_BITS == 8 else 505.0  # full-scale w/ fp16 headroom
OUT_OFF = 128.5 if OUT_BITS == 8 else 512.5     # trunc-to-round bias + offset


def _build_table(coeff):
    """fp16 table [TROWS, TREC]; row (qu,qv,qw) = coeff[:, 2qu:2qu+4, ...]
    laid out [c, x, y, z] in the first 192 elems. Element 192 holds the
    per-tile output quantization gain A = OUT_MARGIN / max|window|."""
    sw = np.lib.stride_tricks.sliding_window_view(coeff, (4, 4, 4), axis=(1, 2, 3))
    sw = sw[:, ::2, ::2, ::2]                        # [3,31,31,31,4,4,4]
    tbl = np.zeros((TROWS, TREC), np.float16)
    payload = np.moveaxis(sw, 0, 3).reshape(TROWS, 192)
    tbl[:, :192] = payload
    S = np.maximum(np.abs(payload).max(axis=1), np.float32(1e-3))
    gain = (np.float32(OUT_MARGIN) / S).astype(np.float16)
    tbl[:, 192] = gain
    return tbl, gain


def _pack_inputs(uvw):
    """-> [Q_chunk0, ...] each [8*3, NB, P, FPTS] u16 (per-core block =
    [3, NB, P, F], plain memcpy of the padded quantized stream),
    tid int32 [8, NP/8] (for output dequant). Per-core work in threads."""
    from concurrent.futures import ThreadPoolExecutor
    Nc = NP_TOTAL // N_CORES
    tot = NCHUNK * CORE_PTS
    chunks = _STATE.get("packbufs")
    if chunks is None:
        chunks = [np.empty((N_CORES * 3, NB, P, FPTS), np.uint16)
                  for _ in range(NCHUNK)]
        _STATE["packbufs"] = chunks

    def pack_core(c):
        t = uvw[:, c * Nc:(c + 1) * Nc] * F32(63488.0)
        np.clip(t, F32(0.0), F32(63487.0), out=t)
        q = t.astype(np.uint16)
        for k in range(NCHUNK):
            dst = chunks[k][c * 3:(c + 1) * 3].reshape(3, CORE_PTS)
            lo = k * CORE_PTS
            n = max(0, min(CORE_PTS, Nc - lo))
            dst[:, :n] = q[:, lo:lo + n]
            if n < CORE_PTS:
                dst[:, n:] = 0

    with ThreadPoolExecutor(N_CORES) as ex:
        list(ex.map(pack_core, range(N_CORES)))
    # tids are derived lazily (inside the D2H-overlapped fetch threads)
    # from the packed q planes; hand back the chunk views needed for it.
    return chunks, None


def _unpack_chunk(out_arr, k, chunks, inv_tab, out, ex):
    """Fetch + dequant one chunk's sharded int32 output into `out`.
    Per-point tile ids (for the dequant gain) are derived here from the
    packed q planes, overlapped with the D2H transfers."""
    Nc = NP_TOTAL // N_CORES
    shards = sorted(out_arr.addressable_shards, key=lambda s: s.index[0].start or 0)

    def fetch(args):
        c, sh = args
        base = k * CORE_PTS
        count = min(CORE_PTS, Nc - base)
        if count <= 0:
            _ = np.asarray(sh.data)  # still drain the transfer
            return
        qv = chunks[k][c * 3:(c + 1) * 3].reshape(3, CORE_PTS)[:, :count]
        tid_c = ((qv[0] >> 11).astype(np.int32) * 961
                 + (qv[1] >> 11).astype(np.int32) * 31
                 + (qv[2] >> 11).astype(np.int32))
        inv = inv_tab[tid_c]
        sl = out[:, c * Nc + base:c * Nc + base + count]
        if OUT_BITS == 8:
            raw = np.asarray(sh.data).reshape(CORE_PTS, 3)[:count]
            v = raw.astype(np.int32) - 128
            sl[0] = v[:, 0].astype(np.float32) * inv
            sl[1] = v[:, 1].astype(np.float32) * inv
            sl[2] = v[:, 2].astype(np.float32) * inv
        else:
            raw = np.asarray(sh.data).reshape(CORE_PTS)[:count]   # i32
            sl[0] = ((raw & 1023) - 512).astype(np.float32) * inv
            sl[1] = (((raw >> 10) & 1023) - 512).astype(np.float32) * inv
            sl[2] = (((raw >> 20) & 1023) - 512).astype(np.float32) * inv

    list(ex.map(fetch, enumerate(shards)))


# ---------------------------------------------------------------------------
# Device program
# ---------------------------------------------------------------------------

def _build_program(table):
    import concourse.bass as bass
    import concourse.tile as tile
    from concourse import bacc, mybir
    from contextlib import ExitStack

    dt = mybir.dt
    op = mybir.AluOpType
    F = FPTS

    nc = bacc.Bacc("TRN2", target_bir_lowering=False, debug=False)
    q_d = nc.dram_tensor("qpk", [3, NB, P, F], dt.uint16, kind="ExternalInput")
    if OUT_BITS == 8:
        o_d = nc.dram_tensor("xyzo", [NB, P, 3 * F], dt.uint8, kind="ExternalOutput")
    else:
        o_d = nc.dram_tensor("xyzo", [NB, P, F], dt.int32, kind="ExternalOutput")
    t_d = nc.inline_tensor(table, name="tbl")

    with nc.allow_low_precision(reason="fp16 partials; scan state is f32"):
        with tile.TileContext(nc) as tc:
            with ExitStack() as ctx:
                cpool = ctx.enter_context(tc.tile_pool(name="c", bufs=1))
                pool = ctx.enter_context(tc.tile_pool(name="p", bufs=2))

                # constant scan mask: repeating [0,1,1,1] fp16
                mask = cpool.tile([P, F * 192], dt.float16, tag="mask")
                nc.vector.memset(mask[:], 1.0)
                nc.vector.memset(
                    mask[:].rearrange("p (s z) -> p s z", z=4)[:, :, 0], 0.0)

                for b in range(NB):
                    qt = pool.tile([P, 3, F], dt.uint16, tag="q")
                    for d in range(3):
                        nc.sync.dma_start(qt[:, d, :], q_d.ap()[d, b])

                    # ---- unpack q -> i, f, du (f32), all 3 dims at once
                    i16 = pool.tile([P, 3, F], dt.uint16, tag="i16")
                    f16i = pool.tile([P, 3, F], dt.uint16, tag="f16i")
                    d16 = pool.tile([P, 3, F], dt.uint16, tag="d16")
                    nc.vector.tensor_scalar(i16[:], qt[:], 10, None, op.logical_shift_right)
                    nc.vector.tensor_scalar(f16i[:], qt[:], 1023, None, op.bitwise_and)
                    nc.vector.tensor_scalar(d16[:], qt[:], 10, 1, op.logical_shift_right, op.bitwise_and)
                    iuf = pool.tile([P, 3, F], dt.float32, tag="iuf")
                    ff = pool.tile([P, 3, F], dt.float32, tag="ff")
                    duf = pool.tile([P, 3, F], dt.float32, tag="duf")
                    nc.vector.tensor_copy(iuf[:], i16[:])
                    nc.vector.tensor_copy(ff[:], f16i[:])
                    nc.vector.tensor_copy(duf[:], d16[:])
                    nc.vector.tensor_scalar(ff[:], ff[:], float(1.0 / 1024.0), None, op.mult)

                    # ---- tile ids (f32 exact) -> int16, shuffled into the
                    # [16, i//16] gather wrap (i = g*128 + p, p = 16j + qq)
                    iu2 = pool.tile([P, 3, F], dt.float32, tag="iu2")
                    nc.vector.tensor_tensor(iu2[:], iuf[:], duf[:], op.subtract)
                    wh = pool.tile([P, F], dt.float32, tag="wh")
                    t1 = pool.tile([P, F], dt.float32, tag="t1")
                    tidf = pool.tile([P, F], dt.float32, tag="tidf")
                    nc.vector.tensor_scalar(wh[:], iu2[:, 2, :], 0.5, None, op.mult)
                    nc.vector.scalar_tensor_tensor(t1[:], iu2[:, 1, :], 15.5, wh[:], op.mult, op.add)
                    nc.vector.scalar_tensor_tensor(tidf[:], iu2[:, 0, :], 480.5, t1[:], op.mult, op.add)
                    tid16 = pool.tile([P, F], dt.int16, tag="tid16")
                    nc.vector.tensor_copy(tid16[:], tidf[:])
                    idx = pool.tile([P, BATCH // 16], dt.int16, tag="idx")
                    idx_v = idx[:].rearrange("p (g j) -> p g j", j=8)
                    for j in range(8):
                        nc.sync.dma_start(idx_v[0:16, :, j], tid16[16 * j:16 * (j + 1), :])
                    for j in range(1, 8):
                        nc.sync.dma_start(idx[16 * j:16 * (j + 1), :], idx[0:16, :])

                    win = pool.tile([P, F, TREC], dt.float16, tag="win")
                    S = GATHER_SPLIT
                    sub = BATCH // S
                    for k in range(S):
                        nc.gpsimd.dma_gather(
                            win[:, k * (F // S):(k + 1) * (F // S), :],
                            t_d.ap(),
                            idx[:, k * (sub // 16):(k + 1) * (sub // 16)],
                            sub, sub, TREC)

                    # ---- basis N0/N1/N2 -> nub [P, 3, F, 5] rows 1..3
                    rd0 = pool.tile([P, 3, F], dt.float32, tag="rd0")
                    rd2 = pool.tile([P, 3, F], dt.float32, tag="rd2")
                    nc.vector.tensor_scalar(rd0[:], iuf[:], 0.0, 0.5, op.is_equal, op.mult)
                    nc.vector.tensor_scalar(rd0[:], rd0[:], 0.5, None, op.add)
                    nc.vector.tensor_scalar(rd2[:], iuf[:], 61.0, 0.5, op.is_equal, op.mult)
                    nc.vector.tensor_scalar(rd2[:], rd2[:], 0.5, None, op.add)
                    omf = pool.tile([P, 3, F], dt.float32, tag="omf")
                    nc.vector.tensor_scalar(omf[:], ff[:], -1.0, 1.0, op.mult, op.add)
                    sq = pool.tile([P, 3, F], dt.float32, tag="sq")
                    nub = pool.tile([P, 3, F, 5], dt.float32, tag="nub")
                    nc.vector.memset(nub[:], 0.0)
                    nc.vector.tensor_tensor(sq[:], omf[:], omf[:], op.mult)
                    nc.vector.tensor_tensor(nub[:, :, :, 1], sq[:], rd0[:], op.mult)
                    nc.vector.tensor_tensor(sq[:], ff[:], ff[:], op.mult)
                    nc.vector.tensor_tensor(nub[:, :, :, 3], sq[:], rd2[:], op.mult)
                    nc.vector.tensor_tensor(sq[:], nub[:, :, :, 1], nub[:, :, :, 3], op.add)
                    nc.vector.tensor_scalar(nub[:, :, :, 2], sq[:], -1.0, 1.0, op.mult, op.add)

                    # ---- extended 4-tap weights nup fp16 [P, 3, F, 4]
                    diff = pool.tile([P, 3, F, 4], dt.float32, tag="diff")
                    nup = pool.tile([P, 3, F, 4], dt.float16, tag="nup")
                    nc.vector.tensor_tensor(diff[:], nub[:, :, :, 0:4], nub[:, :, :, 1:5], op.subtract)
                    nc.vector.tensor_tensor(diff[:], diff[:], duf[:].broadcast_to([P, 3, F, 4]), op.mult)
                    nc.vector.tensor_tensor(nup[:], diff[:], nub[:, :, :, 1:5], op.add)

                    # ---- contraction: [c,x,y,z] win * nw -> scan z -> * nv
                    #      -> scan y -> * nu -> scan x
                    prodz = pool.tile([P, F * 192], dt.float16, tag="prodz")
                    nc.vector.tensor_tensor(
                        prodz[:].rearrange("p (f s z) -> p f s z", s=48, z=4),
                        win[:].rearrange("p f (s z) -> p f s z", z=4)[:, :, 0:48, :],
                        nup[:, 2, :, :].unsqueeze(2).broadcast_to([P, F, 48, 4]),
                        op.mult)
                    nc.vector.tensor_tensor_scan(
                        prodz[:], mask[:], prodz[:], 0.0, op.mult, op.add)
                    prody = pool.tile([P, F * 48], dt.float16, tag="prody")
                    prody_v = prody[:].rearrange("p (f c x y) -> p f c x y",
                                                 c=3, x=4, y=4)
                    zscan_v = prodz[:].rearrange("p (f c x y z) -> p f c x y z",
                                                 c=3, x=4, y=4, z=4)
                    for cc in range(3):
                        nc.vector.tensor_tensor(
                            prody_v[:, :, cc, :, :],
                            zscan_v[:, :, cc, :, :, 3],
                            nup[:, 1, :, :].unsqueeze(2).broadcast_to([P, F, 4, 4]),
                            op.mult)
                    nc.vector.tensor_tensor_scan(
                        prody[:], mask[:, 0:F * 48], prody[:], 0.0, op.mult, op.add)
                    prodx = pool.tile([P, F * 12], dt.float16, tag="prodx")
                    nc.vector.tensor_tensor(
                        prodx[:].rearrange("p (f c x) -> p f c x", c=3, x=4),
                        prody[:].rearrange("p (f c x y) -> p f c x y",
                                           c=3, x=4, y=4)[:, :, :, :, 3],
                        nup[:, 0, :, :].unsqueeze(2).broadcast_to([P, F, 3, 4]),
                        op.mult)
                    nc.vector.tensor_tensor_scan(
                        prodx[:], mask[:, 0:F * 12], prodx[:], 0.0, op.mult, op.add)
                    # ---- fixed-point pack: v = trunc(x*A + OUT_OFF)
                    af = pool.tile([P, F], dt.float32, tag="af")
                    nc.vector.tensor_copy(af[:], win[:, :, 192])
                    xyzf = pool.tile([P, F, 3], dt.float32, tag="xyzf")
                    nc.vector.tensor_copy(
                        xyzf[:],
                        prodx[:].rearrange("p (f c x) -> p f c x", c=3, x=4)[:, :, :, 3])
                    nc.vector.tensor_tensor(
                        xyzf[:], xyzf[:], af[:].broadcast_to([P, F, 3]), op.mult)
                    if OUT_BITS == 8:
                        v8 = pool.tile([P, F * 3], dt.uint8, tag="v8")
                        nc.vector.tensor_scalar(
                            v8[:].rearrange("p (f c) -> p f c", c=3),
                            xyzf[:], OUT_OFF, None, op.add)
                        nc.sync.dma_start(o_d.ap()[b], v8[:])
                    else:
                        vi = pool.tile([P, F, 3], dt.int32, tag="vi")
                        nc.vector.tensor_scalar(vi[:], xyzf[:], OUT_OFF, None, op.add)
                        s1 = pool.tile([P, F], dt.int32, tag="s1")
                        s2 = pool.tile([P, F], dt.int32, tag="s2")
                        nc.vector.tensor_scalar(s1[:], vi[:, :, 1], 10, None, op.logical_shift_left)
                        nc.vector.tensor_scalar(s2[:], vi[:, :, 2], 20, None, op.logical_shift_left)
                        out32 = pool.tile([P, F], dt.int32, tag="out32")
                        nc.vector.tensor_tensor(out32[:], vi[:, :, 0], s1[:], op.bitwise_or)
                        nc.vector.tensor_tensor(out32[:], out32[:], s2[:], op.bitwise_or)
                        nc.sync.dma_start(o_d.ap()[b], out32[:])

    nc.compile()
    return nc


# ---------------------------------------------------------------------------
# Dispatch (cached jitted shard_map over 8 cores)
# ---------------------------------------------------------------------------

_STATE = {"key": None, "fn": None, "fails": 0}
_MAX_DEVICE_FAILS = 3  # transient tunnel/NRT errors shouldn't permanently
                       # demote the process to the slow host path


def _make_dispatch(nc):
    import jax
    import jax.numpy as jnp
    from jax.sharding import Mesh, PartitionSpec
    from jax.experimental.shard_map import shard_map
    from concourse import mybir
    from concourse.bass2jax import (_bass_exec_p, partition_id_tensor,
                                    install_neuronx_cc_hook)

    install_neuronx_cc_hook()
    try:
        jax.config.update("jax_compilation_cache_dir", "/tmp/jax_comp_cache")
        jax.config.update("jax_persistent_cache_min_compile_time_secs", 1.0)
    except Exception:
        pass

    in_names, out_names, out_avals = [], [], []
    partition_name = (nc.partition_id_tensor.name
                      if nc.partition_id_tensor is not None else None)
    for alloc in nc.m.functions[0].allocations:
        if not isinstance(alloc, mybir.MemoryLocationSet):
            continue
        name = alloc.memorylocations[0].name
        if alloc.kind == "ExternalInput":
            if name != partition_name:
                in_names.append(name)
        elif alloc.kind == "ExternalOutput":
            out_names.append(name)
            out_avals.append(jax.core.ShapedArray(
                tuple(alloc.tensor_shape), mybir.dt.np(alloc.dtype)))
    n_params = len(in_names)
    all_names = list(in_names) + list(out_names)
    if partition_name is not None:
        all_names.append(partition_name)

    def _body(*args):
        operands = list(args)
        if partition_name is not None:
            operands.append(partition_id_tensor())
        outs = _bass_exec_p.bind(
            *operands,
            out_avals=tuple(out_avals),
            in_names=tuple(all_names),
            out_names=tuple(out_names),
            lowering_input_output_aliases=(),
            sim_require_finite=False,
            sim_require_nnan=False,
            nc=nc,
        )
        return tuple(outs)

    n_out = len(out_names)
    devices = jax.devices()[:N_CORES]
    mesh = Mesh(np.asarray(devices), ("core",))
    sharding = jax.sharding.NamedSharding(mesh, PartitionSpec("core"))
    fn = jax.jit(shard_map(
        _body, mesh=mesh,
        in_specs=(PartitionSpec("core"),) * (n_params + n_out),
        out_specs=(PartitionSpec("core"),) * n_out,
        check_rep=False),
        donate_argnums=tuple(range(n_params, n_params + n_out)))

    def make_outbufs():
        return [jax.device_put(
            np.zeros(tuple([N_CORES * av.shape[0]] + list(av.shape[1:])),
                     av.dtype), sharding)
                for av in out_avals]

    return fn, in_names, out_names, make_outbufs


def _get_dispatch(coeff):
    key = hash(coeff.tobytes())
    if _STATE["key"] == key:
        return _STATE["fn"]
    table, gain = _build_table(coeff)
    nc = _build_program(table)
    fn = _make_dispatch(nc)
    _STATE["key"] = key
    _STATE["fn"] = fn
    _STATE["nc"] = nc
    _STATE["inv_tab"] = (np.float32(1.0)
                         / gain.astype(np.float32)).astype(np.float32)
    _STATE["outbufs"] = None
    return fn


# ---------------------------------------------------------------------------
# Host fallback (matches reference to ~1e-5)
# ---------------------------------------------------------------------------

def _basis_f32(X):
    X = np.maximum(X, F32(1e-14)).astype(F32)
    t = (X * F32(62.0)).astype(F32)
    C = F32(2 ** 23)
    r = ((t + C) - C).astype(F32)
    g = (t > r).astype(F32)
    i = (r + g - F32(1.0)).astype(F32)
    np.clip(i, F32(0.0), F32(61.0), out=i)
    f = (t - i).astype(F32)
    omf = (F32(1.0) - f).astype(F32)
    eq0 = (i == F32(0.0)).astype(F32)
    eq61 = (i == F32(61.0)).astype(F32)
    rD31 = (eq0 * F32(0.5) + F32(0.5)).astype(F32)
    rD42 = (eq61 * F32(0.5) + F32(0.5)).astype(F32)
    N0 = (omf * omf * rD31).astype(F32)
    N2 = (f * f * rD42).astype(F32)
    N1 = ((F32(1.0) - N0) - N2).astype(F32)
    return i.astype(np.int64), N0, N1, N2


def _spline_eval(uvw, coeff, chunk=262144):
    iu, NU0, NU1, NU2 = _basis_f32(uvw[0])
    iv, NV0, NV1, NV2 = _basis_f32(uvw[1])
    iw, NW0, NW1, NW2 = _basis_f32(uvw[2])
    NU = (NU0, NU1, NU2)
    NV = (NV0, NV1, NV2)
    NW = (NW0, NW1, NW2)
    cf = np.ascontiguousarray(coeff.reshape(3, -1))
    V = np.lib.stride_tricks.sliding_window_view(cf, 3, axis=1)
    base = (iu.astype(np.int32) * np.int32(NGRID * NGRID)
            + iv.astype(np.int32) * np.int32(NGRID) + iw.astype(np.int32))
    N = uvw.shape[1]
    out = np.empty((3, N), dtype=F32)
    for s in range(0, N, chunk):
        e = min(s + chunk, N)
        b = base[s:e]
        acc = np.zeros((3, e - s), dtype=F32)
        for ii in range(3):
            for jj in range(3):
                idx = b + np.int32(ii * NGRID * NGRID + jj * NGRID)
                G = V[:, idx, :]
                wuv = NU[ii][s:e] * NV[jj][s:e]
                w0 = wuv * NW[0][s:e]
                w1 = wuv * NW[1][s:e]
                w2 = wuv * NW[2][s:e]
                acc += G[:, :, 0] * w0 + G[:, :, 1] * w1 + G[:, :, 2] * w2
        out[:, s:e] = acc
    return out


# ---------------------------------------------------------------------------
# Entry point
# ---------------------------------------------------------------------------

def kernel(uvw, knotx, knoty, knotz, coeff, order):
    uvw = np.asarray(uvw, dtype=np.float32)
    coeff = np.asarray(coeff, dtype=np.float32)
    if _STATE["fails"] < _MAX_DEVICE_FAILS:
        try:
            from concurrent.futures import ThreadPoolExecutor
            fn, in_names, out_names, make_outbufs = _get_dispatch(coeff)
            chunks, _ = _pack_inputs(uvw)
            outbufs = _STATE.get("outbufs")
            if outbufs is None:
                outbufs = [make_outbufs() for _ in range(NCHUNK)]
            # dispatch all chunks asynchronously, then drain in order so
            # chunk k's D2H overlaps chunk k+1's H2D/exec
            outs = [fn(chunks[k], *outbufs[k]) for k in range(NCHUNK)]
            _STATE["outbufs"] = [list(o) for o in outs]
            oi = out_names.index("xyzo")
            out = np.empty((3, NP_TOTAL), np.float32)
            with ThreadPoolExecutor(N_CORES) as ex:
                for k in range(NCHUNK):
                    _unpack_chunk(outs[k][oi], k, chunks,
                                  _STATE["inv_tab"], out, ex)
            return out
        except Exception:
            import traceback
            traceback.print_exc()
            _STATE["fails"] += 1
            _STATE["outbufs"] = None
    return _spline_eval(uvw, coeff).astype(np.float32)
